# revision 1
# baseline (speedup 1.0000x reference)
"""Causal self-attention (B=2, T=2048, C=1024, H=16, D=64) on 8 trn2 NeuronCores.

Sharding: core c = (batch b = c//4) x (head-group g = c%4; heads 4g..4g+3).
Tensor-parallel on heads for qkv_proj (column split) / out_proj (row split),
data-parallel on batch. Each core computes a full [C, T] partial of the
output projection for its batch; the host sums the 4 head-group partials
per batch and transposes back to [T, C].

Device kernel (per core), all matmul operands float32r (tf32-like):
  1. QKV: W-stationary matmuls -> Q^T, K^T, V^T in [n, t] layout (+bias).
  2. PE-transpose V^T -> V_aug tiles [t_tile][128, 4*(64+1)] (ones column
     appended per head for the softmax row-sum).
  3. Attention in S^T layout: S^T tile = K_h @ Q_h^T (128tk x 512tq),
     exp(x/8) on ACT, causal mask multiply on the 4 diagonal-crossing
     tiles, then U^T += V_aug^T @ P^T accumulated over tk tiles; row 64 of
     the psum accumulates the softmax denominator l.
  4. Normalize: r = 1/l (DVE), partition-broadcast r (GPSIMD), multiply.
  5. Out-proj: W_out-stationary matmuls -> out^T [1024, 2048] + bias.
"""

import sys

if "/opt/trn_rl_repo" not in sys.path:
    sys.path.insert(0, "/opt/trn_rl_repo")

import numpy as np

B, T, C = 2, 2048, 1024
H, D = 16, 64
HPC = 4            # heads per core
NC_ = HPC * D      # 256 qkv columns per core per projection
N_CORES = 8
PT = 128           # partition tile
TT = T // PT       # 16 t tiles
QC = 512           # tq chunk (moving free dim)
NQC = T // QC      # 4 tq chunks
KC = C // PT       # 8 contraction chunks for qkv proj
MQKV = 3 * NC_ // PT  # 6 output row chunks of qkv proj
VA = D + 1         # v_aug cols per head

_CACHE = {}
_DEBUG = False


def _build_nc():
    import concourse.bacc as bacc
    import concourse.mybir as mybir
    import concourse.tile as tile
    from contextlib import ExitStack

    f32 = mybir.dt.float32
    f32r = mybir.dt.float32r
    Act = mybir.ActivationFunctionType

    nc = bacc.Bacc("TRN2", target_bir_lowering=False, debug=False,
                   num_devices=N_CORES)

    xT_d = nc.dram_tensor("xT", [C, T], f32, kind="ExternalInput").ap()
    wqkv_d = nc.dram_tensor("wqkv", [C, 3 * NC_], f32, kind="ExternalInput").ap()
    bqkv_d = nc.dram_tensor("bqkv", [3 * NC_, 1], f32, kind="ExternalInput").ap()
    wout_d = nc.dram_tensor("wout", [NC_, C], f32, kind="ExternalInput").ap()
    bout_d = nc.dram_tensor("bout", [C, 1], f32, kind="ExternalInput").ap()
    trimask_d = nc.dram_tensor("trimask", [PT, PT], f32, kind="ExternalInput").ap()
    ones4_d = nc.dram_tensor("ones4", [PT, HPC], f32, kind="ExternalInput").ap()
    ident_d = nc.dram_tensor("ident", [PT, PT], f32, kind="ExternalInput").ap()
    outT_d = nc.dram_tensor("outT", [C, T], f32, kind="ExternalOutput").ap()
    if _DEBUG:
        dbgqk_d = nc.dram_tensor("dbgqk", [4 * PT, T], f32, kind="ExternalOutput").ap()
        dbgva_d = nc.dram_tensor("dbgva", [TT * PT, HPC * VA], f32, kind="ExternalOutput").ap()
        dbgun_d = nc.dram_tensor("dbgun", [2 * PT, T], f32, kind="ExternalOutput").ap()

    with tile.TileContext(nc) as tc, ExitStack() as ctx:
        p_xt = ctx.enter_context(tc.tile_pool(name="xt", bufs=KC))
        p_wq = ctx.enter_context(tc.tile_pool(name="wq", bufs=KC))
        p_qk = ctx.enter_context(tc.tile_pool(name="qk", bufs=4))
        p_vt = ctx.enter_context(tc.tile_pool(name="vt", bufs=4))
        p_va = ctx.enter_context(tc.tile_pool(name="va", bufs=TT))
        p_wo = ctx.enter_context(tc.tile_pool(name="wo", bufs=2))
        p_small = ctx.enter_context(tc.tile_pool(name="small", bufs=1))
        p_pt = ctx.enter_context(tc.tile_pool(name="ptile", bufs=3))
        p_norm = ctx.enter_context(tc.tile_pool(name="norm", bufs=2))
        p_out = ctx.enter_context(tc.tile_pool(name="outs", bufs=2))
        ps_1 = ctx.enter_context(tc.tile_pool(name="ps1", bufs=2, space="PSUM"))
        ps_2 = ctx.enter_context(tc.tile_pool(name="ps2", bufs=2, space="PSUM"))

        # ---- loads -------------------------------------------------------
        wq_sb = []
        for k in range(KC):
            w = p_wq.tile([PT, 3 * NC_], f32r, tag="wq")
            nc.sync.dma_start(w[:], wqkv_d[k * PT:(k + 1) * PT, :].bitcast(f32r))
            wq_sb.append(w)

        bq_sb = []
        for m in range(MQKV):
            b = p_small.tile([PT, 1], f32, tag="bq", bufs=MQKV)
            nc.sync.dma_start(b[:], bqkv_d[m * PT:(m + 1) * PT, :])
            bq_sb.append(b)

        # x^T column-chunked loads so qkv matmuls can start early
        xt_sb = [p_xt.tile([PT, T], f32r, tag="xt", name=f"xt{k}") for k in range(KC)]
        for cq in range(NQC):
            for k in range(KC):
                nc.sync.dma_start(
                    xt_sb[k][:, cq * QC:(cq + 1) * QC],
                    xT_d[k * PT:(k + 1) * PT, cq * QC:(cq + 1) * QC].bitcast(f32r),
                )

        trimask = p_small.tile([PT, PT], f32r, tag="trimask")
        nc.sync.dma_start(trimask[:], trimask_d[:].bitcast(f32r))
        ident = p_small.tile([PT, PT], f32r, tag="ident")
        nc.sync.dma_start(ident[:], ident_d[:].bitcast(f32r))
        wo_sb = []
        for k in range(2):
            w = p_wo.tile([PT, C], f32r, tag="wo")
            nc.sync.dma_start(w[:], wout_d[k * PT:(k + 1) * PT, :].bitcast(f32r))
            wo_sb.append(w)
        bo_sb = []
        for e in range(C // PT):
            b = p_small.tile([PT, 1], f32, tag="bo", bufs=C // PT)
            nc.sync.dma_start(b[:], bout_d[e * PT:(e + 1) * PT, :])
            bo_sb.append(b)

        # ---- phase 1: qkv projection -> Q^T, K^T (packed), V_aug --------
        # m chunk 0..1 -> Q^T rows, 2..3 -> K^T rows, 4..5 -> V^T rows
        qk_sb = [p_qk.tile([PT, T], f32r, tag="qk", name=f"qk{j}") for j in range(4)]
        va_sb = [p_va.tile([PT, HPC * VA], f32r, tag="va", name=f"va{t}") for t in range(TT)]

        for cq in range(NQC):
            cs = slice(cq * QC, (cq + 1) * QC)
            for m in range(MQKV):
                ps = ps_1.tile([PT, QC], f32, tag="sa")
                for k in range(KC):
                    nc.tensor.matmul(
                        ps[:],
                        wq_sb[k][:, m * PT:(m + 1) * PT],
                        xt_sb[k][:, cs],
                        start=(k == 0), stop=(k == KC - 1),
                    )
                if m < 4:
                    nc.vector.tensor_scalar_add(qk_sb[m][:, cs], ps[:], bq_sb[m])
                else:
                    h0, h1 = 2 * (m - 4), 2 * (m - 4) + 1
                    for q4 in range(4):
                        t = cq * 4 + q4
                        vtp = p_vt.tile([PT, PT], f32r, tag="vt")
                        nc.vector.tensor_scalar_add(
                            vtp[:], ps[:, q4 * PT:(q4 + 1) * PT], bq_sb[m])
                        pst = ps_1.tile([PT, PT], f32r, tag="sa")
                        nc.tensor.transpose(pst[:], vtp[:], ident[:])
                        nc.vector.tensor_copy(
                            va_sb[t][:, h0 * VA:h0 * VA + D], pst[:, 0:D])
                        nc.vector.tensor_copy(
                            va_sb[t][:, h1 * VA:h1 * VA + D], pst[:, D:2 * D])

        # ones columns for the softmax row-sum (gpsimd queue; off the SP path)
        for t in range(TT):
            nc.gpsimd.dma_start(
                va_sb[t].rearrange("p (h v) -> p h v", v=VA)[:, :, D:D + 1],
                ones4_d[:].bitcast(f32r),
            )

        # ---- phase 2+3: attention (head pairs) + interleaved out-proj ----
        # UN tiles reuse xt pool slots (x^T fully consumed by phase 1)
        un_sb = [p_xt.tile([PT, T], f32r, tag="xt", name=f"un{j}") for j in range(2)]

        for cq in range(NQC):
            cs = slice(cq * QC, (cq + 1) * QC)
            nts = 4 * cq + 4
            for j in range(2):          # head pair (2j, 2j+1)
                h0, h1 = 2 * j, 2 * j + 1
                psu = ps_2.tile([PT, 2 * QC], f32, tag="acc")
                for t in range(nts):
                    p = t - 4 * cq      # >= 0 on diagonal-crossing tiles
                    s = max(p, 0) * PT  # skip fully-masked leading columns
                    w = QC - s
                    psS = ps_1.tile([PT, 2 * QC], f32, tag="sa")
                    tsl = slice(t * PT, (t + 1) * PT)
                    qsl = slice(cq * QC + s, (cq + 1) * QC)
                    nc.tensor.matmul(
                        psS[:, s:QC],
                        qk_sb[2 + j][0:D, tsl], qk_sb[j][0:D, qsl],
                        start=True, stop=True, tile_position=(0, 0),
                    )
                    nc.tensor.matmul(
                        psS[:, QC + s:2 * QC],
                        qk_sb[2 + j][D:PT, tsl], qk_sb[j][D:PT, qsl],
                        start=True, stop=True, tile_position=(D, 0),
                    )
                    pt = p_pt.tile([PT, 2 * QC], f32r, tag="pt")
                    pt3 = pt.rearrange("p (h w) -> p h w", h=2)
                    psS3 = psS.rearrange("p (h w) -> p h w", h=2)
                    nc.scalar.activation(pt3[:, :, s:QC], psS3[:, :, s:QC],
                                         Act.Exp, scale=0.125)
                    if p >= 0:
                        nc.vector.tensor_mul(
                            pt[:, s:s + PT], pt[:, s:s + PT], trimask[:])
                        nc.vector.tensor_mul(
                            pt[:, QC + s:QC + s + PT], pt[:, QC + s:QC + s + PT],
                            trimask[:])
                    nc.tensor.matmul(
                        psu[0:VA, s:QC],
                        va_sb[t][:, h0 * VA:(h0 + 1) * VA], pt[:, s:QC],
                        start=(t == 0), stop=(t == nts - 1),
                    )
                    nc.tensor.matmul(
                        psu[0:VA, QC + s:2 * QC],
                        va_sb[t][:, h1 * VA:(h1 + 1) * VA], pt[:, QC + s:2 * QC],
                        start=(t == 0), stop=(t == nts - 1),
                    )
                # normalize: rows 0..63 = U^T, row 64 = l (both heads)
                rr = p_norm.tile([VA, 2 * QC], f32, tag="rr")
                nc.vector.reciprocal(rr[D:VA, :], psu[D:VA, :])
                rb = p_norm.tile([D, 2 * QC], f32, tag="rb")
                # shift 1/l from partition 64 to partition 0, then broadcast
                # (partition_broadcast ucode reads physical partition 0)
                nc.gpsimd.dma_start(rb[0:1, :], rr[D:VA, :])
                nc.gpsimd.partition_broadcast(rb[0:D, :], rb[0:1, :])
                nc.vector.tensor_mul(un_sb[j][0:D, cs], psu[0:D, 0:QC],
                                     rb[:, 0:QC])
                ut = p_norm.tile([D, QC], f32r, tag="ut")
                nc.vector.tensor_mul(ut[:], psu[0:D, QC:2 * QC], rb[:, QC:2 * QC])
                nc.gpsimd.dma_start(un_sb[j][D:PT, cs], ut[:])

            # out-projection for this tq chunk (e-chunk pairs)
            for ep in range(4):
                pp2 = ps_1.tile([PT, 2 * QC], f32, tag="sa")
                for half in range(2):
                    e = 2 * ep + half
                    for k in range(2):
                        nc.tensor.matmul(
                            pp2[:, half * QC:(half + 1) * QC],
                            wo_sb[k][:, e * PT:(e + 1) * PT],
                            un_sb[k][:, cs],
                            start=(k == 0), stop=(k == 1),
                        )
                ot = p_out.tile([PT, 2 * QC], f32, tag="ot")
                for half in range(2):
                    e = 2 * ep + half
                    nc.vector.tensor_scalar_add(
                        ot[:, half * QC:(half + 1) * QC],
                        pp2[:, half * QC:(half + 1) * QC], bo_sb[e])
                    nc.sync.dma_start(
                        outT_d[e * PT:(e + 1) * PT, cs],
                        ot[:, half * QC:(half + 1) * QC])

        if _DEBUG:
            for j in range(4):
                nc.sync.dma_start(dbgqk_d[j * PT:(j + 1) * PT, :],
                                  qk_sb[j][:].bitcast(f32))
            for t in range(TT):
                nc.sync.dma_start(dbgva_d[t * PT:(t + 1) * PT, :],
                                  va_sb[t][:].bitcast(f32))
            for j in range(2):
                nc.sync.dma_start(dbgun_d[j * PT:(j + 1) * PT, :],
                                  un_sb[j][:].bitcast(f32))

    nc.compile()
    return nc


def _get_nc():
    if "nc" not in _CACHE:
        _CACHE["nc"] = _build_nc()
    return _CACHE["nc"]


def _make_in_maps(x, W_qkv, b_qkv, W_out, b_out):
    x = np.asarray(x, dtype=np.float32)
    W_qkv = np.asarray(W_qkv, dtype=np.float32)
    b_qkv = np.asarray(b_qkv, dtype=np.float32)
    W_out = np.asarray(W_out, dtype=np.float32)
    b_out = np.asarray(b_out, dtype=np.float32)

    i = np.arange(PT)[:, None]
    j = np.arange(PT)[None, :]
    trimask = (i <= j).astype(np.float32)
    ones4 = np.ones((PT, HPC), dtype=np.float32)
    ident = np.eye(PT, dtype=np.float32)

    in_maps = []
    for c in range(N_CORES):
        b, g = divmod(c, 4)
        gs = slice(g * NC_, (g + 1) * NC_)
        wqkv_c = np.ascontiguousarray(np.concatenate(
            [W_qkv[:, gs], W_qkv[:, C:][:, gs], W_qkv[:, 2 * C:][:, gs]],
            axis=1))
        bqkv_c = np.ascontiguousarray(np.concatenate(
            [b_qkv[gs], b_qkv[C:][gs], b_qkv[2 * C:][gs]])[:, None])
        bout_c = (b_out if g == 0 else np.zeros_like(b_out))[:, None]
        in_maps.append({
            "xT": np.ascontiguousarray(x[b].T),
            "wqkv": wqkv_c,
            "bqkv": bqkv_c,
            "wout": np.ascontiguousarray(W_out[gs, :]),
            "bout": np.ascontiguousarray(bout_c),
            "trimask": trimask,
            "ones4": ones4,
            "ident": ident,
        })
    return in_maps


def _assemble(results):
    out = np.empty((B, T, C), dtype=np.float32)
    for b in range(B):
        acc = results[4 * b]["outT"].copy()
        for g in range(1, 4):
            acc += results[4 * b + g]["outT"]
        out[b] = acc.T
    return out


def kernel(x, W_qkv, b_qkv, W_out, b_out):
    from concourse import bass_utils
    nc = _get_nc()
    in_maps = _make_in_maps(x, W_qkv, b_qkv, W_out, b_out)
    res = bass_utils.run_bass_kernel_spmd(nc, in_maps, core_ids=list(range(N_CORES)))
    return _assemble(res.results)



# revision 3
# speedup vs baseline: 1.3058x; 1.3058x over previous
"""Causal self-attention (B=2, T=2048, C=1024, H=16, D=64) on 8 trn2 NeuronCores.

Sharding: core c = (batch b = c//4) x (head-group g = c%4; heads 4g..4g+3).
Tensor-parallel on heads for qkv_proj (column split) / out_proj (row split),
data-parallel on batch. Each core computes a full [C, T] partial of the
output projection for its batch; the host sums the 4 head-group partials
per batch and transposes back to [T, C].

Device kernel (per core), all matmul operands bf16 (psum accumulate f32):
  Fused tq-chunk pipeline, per cq (512 tokens):
  1. QKV(cq): W-stationary matmuls -> Q^T, K^T cols (+bias), V^T ->
     PE-transpose -> V_aug tiles [128, 4*(64+1)] (ones column per head,
     DMA'd once at start, for the softmax row-sum).
  2. Attention(cq) in S^T layout per head pair: S^T tile = K_h @ Q_h^T
     (two heads packed in PE row strips), exp(x/8) on ACT, causal mask
     multiply on diagonal tiles, U^T += V_aug^T @ P^T over tk tiles;
     psum row 64 accumulates the denominator l. Normalize via DVE
     reciprocal + gpsimd partition-broadcast.
  3. Out-proj(cq-1): W_out-stationary matmuls on the previous chunk's
     normalized U^T (software-pipelined so the PE never waits on the
     normalize chain).
"""

import sys

if "/opt/trn_rl_repo" not in sys.path:
    sys.path.insert(0, "/opt/trn_rl_repo")

import numpy as np

B, T, C = 2, 2048, 1024
H, D = 16, 64
HPC = 4            # heads per core
NC_ = HPC * D      # 256 qkv columns per core per projection
N_CORES = 8
PT = 128           # partition tile
TT = T // PT       # 16 t tiles
QC = 512           # tq chunk (moving free dim)
NQC = T // QC      # 4 tq chunks
KC = C // PT       # 8 contraction chunks for qkv proj
MQKV = 3 * NC_ // PT  # 6 output row chunks of qkv proj
VA = D + 1         # v_aug cols per head

_CACHE = {}


def _build_nc():
    import concourse.bacc as bacc
    import concourse.mybir as mybir
    import concourse.tile as tile
    from contextlib import ExitStack

    f32 = mybir.dt.float32
    bf16 = mybir.dt.bfloat16
    Act = mybir.ActivationFunctionType

    nc = bacc.Bacc("TRN2", target_bir_lowering=False, debug=False,
                   num_devices=N_CORES)

    xT_d = nc.dram_tensor("xT", [C, T], bf16, kind="ExternalInput").ap()
    wqkv_d = nc.dram_tensor("wqkv", [C, 3 * NC_], bf16, kind="ExternalInput").ap()
    bqkv_d = nc.dram_tensor("bqkv", [3 * NC_, 1], f32, kind="ExternalInput").ap()
    wout_d = nc.dram_tensor("wout", [NC_, C], bf16, kind="ExternalInput").ap()
    bout_d = nc.dram_tensor("bout", [C, 1], f32, kind="ExternalInput").ap()
    trimask_d = nc.dram_tensor("trimask", [PT, PT], bf16, kind="ExternalInput").ap()
    ones4_d = nc.dram_tensor("ones4", [PT, HPC], bf16, kind="ExternalInput").ap()
    ident_d = nc.dram_tensor("ident", [PT, PT], bf16, kind="ExternalInput").ap()
    outT_d = nc.dram_tensor("outT", [C, T], f32, kind="ExternalOutput").ap()

    with tile.TileContext(nc) as tc, ExitStack() as ctx:
        p_xt = ctx.enter_context(tc.tile_pool(name="xt", bufs=KC))
        p_wq = ctx.enter_context(tc.tile_pool(name="wq", bufs=KC))
        p_qk = ctx.enter_context(tc.tile_pool(name="qk", bufs=4))
        p_vt = ctx.enter_context(tc.tile_pool(name="vt", bufs=4))
        p_va = ctx.enter_context(tc.tile_pool(name="va", bufs=TT))
        p_wo = ctx.enter_context(tc.tile_pool(name="wo", bufs=2))
        p_un = ctx.enter_context(tc.tile_pool(name="un", bufs=2))
        p_small = ctx.enter_context(tc.tile_pool(name="small", bufs=1))
        p_pt = ctx.enter_context(tc.tile_pool(name="ptile", bufs=3))
        p_norm = ctx.enter_context(tc.tile_pool(name="norm", bufs=2))
        p_out = ctx.enter_context(tc.tile_pool(name="outs", bufs=2))
        ps_1 = ctx.enter_context(tc.tile_pool(name="ps1", bufs=2, space="PSUM"))
        ps_2 = ctx.enter_context(tc.tile_pool(name="ps2", bufs=2, space="PSUM"))

        # ---- loads -------------------------------------------------------
        wq_sb = []
        for k in range(KC):
            w = p_wq.tile([PT, 3 * NC_], bf16, tag="wq")
            nc.sync.dma_start(w[:], wqkv_d[k * PT:(k + 1) * PT, :])
            wq_sb.append(w)

        bq_sb = []
        for m in range(MQKV):
            b = p_small.tile([PT, 1], f32, tag="bq", bufs=MQKV)
            nc.sync.dma_start(b[:], bqkv_d[m * PT:(m + 1) * PT, :])
            bq_sb.append(b)

        # x^T column-chunked loads so qkv matmuls can start early
        xt_sb = [p_xt.tile([PT, T], bf16, tag="xt", name=f"xt{k}") for k in range(KC)]
        for cq in range(NQC):
            for k in range(KC):
                nc.sync.dma_start(
                    xt_sb[k][:, cq * QC:(cq + 1) * QC],
                    xT_d[k * PT:(k + 1) * PT, cq * QC:(cq + 1) * QC],
                )

        trimask = p_small.tile([PT, PT], bf16, tag="trimask")
        nc.sync.dma_start(trimask[:], trimask_d[:])
        ident = p_small.tile([PT, PT], bf16, tag="ident")
        nc.sync.dma_start(ident[:], ident_d[:])
        wo_sb = []
        for k in range(2):
            w = p_wo.tile([PT, C], bf16, tag="wo")
            nc.sync.dma_start(w[:], wout_d[k * PT:(k + 1) * PT, :])
            wo_sb.append(w)
        bo_sb = []
        for e in range(C // PT):
            b = p_small.tile([PT, 1], f32, tag="bo", bufs=C // PT)
            nc.sync.dma_start(b[:], bout_d[e * PT:(e + 1) * PT, :])
            bo_sb.append(b)

        qk_sb = [p_qk.tile([PT, T], bf16, tag="qk", name=f"qk{j}") for j in range(4)]
        va_sb = [p_va.tile([PT, HPC * VA], bf16, tag="va", name=f"va{t}") for t in range(TT)]
        un_sb = [p_un.tile([PT, T], bf16, tag="un", name=f"un{j}") for j in range(2)]

        # ones columns for the softmax row-sum, filled once up front
        # (transposed V only ever writes the D v-columns per head)
        for t in range(TT):
            nc.gpsimd.dma_start(
                va_sb[t].rearrange("p (h v) -> p h v", v=VA)[:, :, D:D + 1],
                ones4_d[:],
            )

        def qkv_chunk(cq):
            cs = slice(cq * QC, (cq + 1) * QC)
            for m in range(MQKV):
                ps = ps_1.tile([PT, QC], f32, tag="sa")
                for k in range(KC):
                    nc.tensor.matmul(
                        ps[:],
                        wq_sb[k][:, m * PT:(m + 1) * PT],
                        xt_sb[k][:, cs],
                        start=(k == 0), stop=(k == KC - 1),
                    )
                if m < 4:
                    nc.vector.tensor_scalar_add(qk_sb[m][:, cs], ps[:], bq_sb[m])
                else:
                    h0, h1 = 2 * (m - 4), 2 * (m - 4) + 1
                    for q4 in range(4):
                        t = cq * 4 + q4
                        vtp = p_vt.tile([PT, PT], bf16, tag="vt")
                        nc.vector.tensor_scalar_add(
                            vtp[:], ps[:, q4 * PT:(q4 + 1) * PT], bq_sb[m])
                        pst = ps_1.tile([PT, PT], bf16, tag="sa")
                        nc.tensor.transpose(pst[:], vtp[:], ident[:])
                        nc.vector.tensor_copy(
                            va_sb[t][:, h0 * VA:h0 * VA + D], pst[:, 0:D])
                        nc.vector.tensor_copy(
                            va_sb[t][:, h1 * VA:h1 * VA + D], pst[:, D:2 * D])

        def attn_chunk(cq):
            cs = slice(cq * QC, (cq + 1) * QC)
            nts = 4 * cq + 4
            for j in range(2):          # head pair (2j, 2j+1)
                h0, h1 = 2 * j, 2 * j + 1
                psu = ps_2.tile([PT, 2 * QC], f32, tag="acc")
                for t in range(nts):
                    p = t - 4 * cq      # >= 0 on diagonal-crossing tiles
                    s = max(p, 0) * PT  # skip fully-masked leading columns
                    w = QC - s
                    psS = ps_1.tile([PT, 2 * QC], f32, tag="sa")
                    tsl = slice(t * PT, (t + 1) * PT)
                    qsl = slice(cq * QC + s, (cq + 1) * QC)
                    nc.tensor.matmul(
                        psS[:, s:QC],
                        qk_sb[2 + j][0:D, tsl], qk_sb[j][0:D, qsl],
                        start=True, stop=True, tile_position=(0, 0),
                    )
                    nc.tensor.matmul(
                        psS[:, QC + s:2 * QC],
                        qk_sb[2 + j][D:PT, tsl], qk_sb[j][D:PT, qsl],
                        start=True, stop=True, tile_position=(D, 0),
                    )
                    pt = p_pt.tile([PT, 2 * QC], bf16, tag="pt")
                    pt3 = pt.rearrange("p (h w) -> p h w", h=2)
                    psS3 = psS.rearrange("p (h w) -> p h w", h=2)
                    nc.scalar.activation(pt3[:, :, s:QC], psS3[:, :, s:QC],
                                         Act.Exp, scale=0.125)
                    if p >= 0:
                        nc.vector.tensor_mul(
                            pt[:, s:s + PT], pt[:, s:s + PT], trimask[:])
                        nc.vector.tensor_mul(
                            pt[:, QC + s:QC + s + PT], pt[:, QC + s:QC + s + PT],
                            trimask[:])
                    nc.tensor.matmul(
                        psu[0:VA, s:QC],
                        va_sb[t][:, h0 * VA:(h0 + 1) * VA], pt[:, s:QC],
                        start=(t == 0), stop=(t == nts - 1),
                    )
                    nc.tensor.matmul(
                        psu[0:VA, QC + s:2 * QC],
                        va_sb[t][:, h1 * VA:(h1 + 1) * VA], pt[:, QC + s:2 * QC],
                        start=(t == 0), stop=(t == nts - 1),
                    )
                # normalize: rows 0..63 = U^T, row 64 = l (both heads)
                rr = p_norm.tile([VA, 2 * QC], f32, tag="rr")
                nc.vector.reciprocal(rr[D:VA, :], psu[D:VA, :])
                rb = p_norm.tile([D, 2 * QC], f32, tag="rb")
                # shift 1/l from partition 64 to partition 0, then broadcast
                # (partition_broadcast ucode reads physical partition 0)
                nc.gpsimd.dma_start(rb[0:1, :], rr[D:VA, :])
                nc.gpsimd.partition_broadcast(rb[0:D, :], rb[0:1, :])
                nc.vector.tensor_mul(un_sb[j][0:D, cs], psu[0:D, 0:QC],
                                     rb[:, 0:QC])
                ut = p_norm.tile([D, QC], bf16, tag="ut")
                nc.vector.tensor_mul(ut[:], psu[0:D, QC:2 * QC], rb[:, QC:2 * QC])
                nc.gpsimd.dma_start(un_sb[j][D:PT, cs], ut[:])

        def outproj_chunk(cq):
            cs = slice(cq * QC, (cq + 1) * QC)
            for ep in range(4):
                pp2 = ps_1.tile([PT, 2 * QC], f32, tag="sa")
                for half in range(2):
                    e = 2 * ep + half
                    for k in range(2):
                        nc.tensor.matmul(
                            pp2[:, half * QC:(half + 1) * QC],
                            wo_sb[k][:, e * PT:(e + 1) * PT],
                            un_sb[k][:, cs],
                            start=(k == 0), stop=(k == 1),
                        )
                ot = p_out.tile([PT, 2 * QC], f32, tag="ot")
                for half in range(2):
                    e = 2 * ep + half
                    nc.vector.tensor_scalar_add(
                        ot[:, half * QC:(half + 1) * QC],
                        pp2[:, half * QC:(half + 1) * QC], bo_sb[e])
                    nc.sync.dma_start(
                        outT_d[e * PT:(e + 1) * PT, cs],
                        ot[:, half * QC:(half + 1) * QC])

        # fused pipeline: out-proj trails attention by one chunk so the
        # PE never stalls on the normalize chain
        for cq in range(NQC):
            qkv_chunk(cq)
            attn_chunk(cq)
            if cq > 0:
                outproj_chunk(cq - 1)
        outproj_chunk(NQC - 1)

    nc.compile()
    return nc


def _get_nc():
    if "nc" not in _CACHE:
        _CACHE["nc"] = _build_nc()
    return _CACHE["nc"]


def _make_in_maps(x, W_qkv, b_qkv, W_out, b_out):
    import ml_dtypes

    bf16 = ml_dtypes.bfloat16
    x = np.asarray(x, dtype=np.float32)
    W_qkv = np.asarray(W_qkv, dtype=np.float32)
    b_qkv = np.asarray(b_qkv, dtype=np.float32)
    W_out = np.asarray(W_out, dtype=np.float32)
    b_out = np.asarray(b_out, dtype=np.float32)

    i = np.arange(PT)[:, None]
    j = np.arange(PT)[None, :]
    trimask = (i <= j).astype(bf16)
    ones4 = np.ones((PT, HPC), dtype=bf16)
    ident = np.eye(PT, dtype=bf16)

    in_maps = []
    for c in range(N_CORES):
        b, g = divmod(c, 4)
        gs = slice(g * NC_, (g + 1) * NC_)
        wqkv_c = np.ascontiguousarray(np.concatenate(
            [W_qkv[:, gs], W_qkv[:, C:][:, gs], W_qkv[:, 2 * C:][:, gs]],
            axis=1).astype(bf16))
        bqkv_c = np.ascontiguousarray(np.concatenate(
            [b_qkv[gs], b_qkv[C:][gs], b_qkv[2 * C:][gs]])[:, None])
        bout_c = (b_out if g == 0 else np.zeros_like(b_out))[:, None]
        in_maps.append({
            "xT": np.ascontiguousarray(x[b].T.astype(bf16)),
            "wqkv": wqkv_c,
            "bqkv": bqkv_c,
            "wout": np.ascontiguousarray(W_out[gs, :].astype(bf16)),
            "bout": np.ascontiguousarray(bout_c),
            "trimask": trimask,
            "ones4": ones4,
            "ident": ident,
        })
    return in_maps


def _assemble(results):
    out = np.empty((B, T, C), dtype=np.float32)
    for b in range(B):
        acc = results[4 * b]["outT"].copy()
        for g in range(1, 4):
            acc += results[4 * b + g]["outT"]
        out[b] = acc.T
    return out


def kernel(x, W_qkv, b_qkv, W_out, b_out):
    from concourse import bass_utils
    nc = _get_nc()
    in_maps = _make_in_maps(x, W_qkv, b_qkv, W_out, b_out)
    res = bass_utils.run_bass_kernel_spmd(nc, in_maps, core_ids=list(range(N_CORES)))
    return _assemble(res.results)


# revision 14
# speedup vs baseline: 1.3472x; 1.0317x over previous
"""Causal self-attention (B=2, T=2048, C=1024, H=16, D=64) on 8 trn2 NeuronCores.

Sharding: core c = (batch b = c//4) x (head-group g = c%4; heads 4g..4g+3).
Tensor-parallel on heads for qkv_proj (column split) / out_proj (row split),
data-parallel on batch. Each core computes a full [C, T] partial of the
output projection for its batch; the host sums the 4 head-group partials
per batch and transposes back to [T, C].

Device kernel (per core), all matmul operands bf16 (psum accumulate f32):
  Fused tq-chunk pipeline, per cq (512 tokens):
  1. QKV(cq): W-stationary matmuls -> Q^T, K^T cols (+bias), V^T ->
     PE-transpose -> V_aug tiles [128, 4*(64+1)] (ones column per head,
     DMA'd once at start, for the softmax row-sum).
  2. Attention(cq) in S^T layout per head pair: S^T tile = K_h @ Q_h^T
     (two heads packed in PE row strips), exp(x/8) on ACT, causal mask
     multiply on diagonal tiles, U^T += V_aug^T @ P^T over tk tiles;
     psum row 64 accumulates the denominator l. Normalize via DVE
     reciprocal + gpsimd partition-broadcast.
  3. Out-proj(cq-1): W_out-stationary matmuls on the previous chunk's
     normalized U^T (software-pipelined so the PE never waits on the
     normalize chain).
"""

import sys

if "/opt/trn_rl_repo" not in sys.path:
    sys.path.insert(0, "/opt/trn_rl_repo")

import numpy as np

B, T, C = 2, 2048, 1024
H, D = 16, 64
HPC = 4            # heads per core
NC_ = HPC * D      # 256 qkv columns per core per projection
N_CORES = 8
PT = 128           # partition tile
TT = T // PT       # 16 t tiles
QC = 512           # tq chunk (moving free dim)
NQC = T // QC      # 4 tq chunks
KC = C // PT       # 8 contraction chunks for qkv proj
MQKV = 3 * NC_ // PT  # 6 output row chunks of qkv proj
VA = D + 1         # v_aug live cols per head (64 V + 1 ones)
VAP = 80           # padded per-head stride in va tiles (32B-aligned for DMA XBAR)

_CACHE = {}


def _build_nc():
    import concourse.bacc as bacc
    import concourse.mybir as mybir
    import concourse.tile as tile
    from contextlib import ExitStack

    f32 = mybir.dt.float32
    bf16 = mybir.dt.bfloat16
    Act = mybir.ActivationFunctionType

    nc = bacc.Bacc("TRN2", target_bir_lowering=False, debug=False,
                   num_devices=N_CORES)

    xT_d = nc.dram_tensor("xT", [C, T], bf16, kind="ExternalInput").ap()
    wqkv_d = nc.dram_tensor("wqkv", [C, 3 * NC_], bf16, kind="ExternalInput").ap()
    bqkv_d = nc.dram_tensor("bqkv", [3 * NC_, 1], f32, kind="ExternalInput").ap()
    wout_d = nc.dram_tensor("wout", [NC_, C], bf16, kind="ExternalInput").ap()
    bout_d = nc.dram_tensor("bout", [C, 1], f32, kind="ExternalInput").ap()
    trimask_d = nc.dram_tensor("trimask", [PT, PT], bf16, kind="ExternalInput").ap()
    ones4_d = nc.dram_tensor("ones4", [PT, HPC], bf16, kind="ExternalInput").ap()
    ident_d = nc.dram_tensor("ident", [PT, PT], bf16, kind="ExternalInput").ap()
    outT_d = nc.dram_tensor("outT", [C, T], bf16, kind="ExternalOutput").ap()

    with tile.TileContext(nc) as tc, ExitStack() as ctx:
        p_xt = ctx.enter_context(tc.tile_pool(name="xt", bufs=KC))
        p_wq = ctx.enter_context(tc.tile_pool(name="wq", bufs=KC))
        p_qk = ctx.enter_context(tc.tile_pool(name="qk", bufs=4))
        p_vt = ctx.enter_context(tc.tile_pool(name="vt", bufs=4))
        p_va = ctx.enter_context(tc.tile_pool(name="va", bufs=TT))
        p_wo = ctx.enter_context(tc.tile_pool(name="wo", bufs=2))
        p_un = ctx.enter_context(tc.tile_pool(name="un", bufs=2))
        p_small = ctx.enter_context(tc.tile_pool(name="small", bufs=1))
        p_pt = ctx.enter_context(tc.tile_pool(name="ptile", bufs=3))
        p_norm = ctx.enter_context(tc.tile_pool(name="norm", bufs=4))
        p_out = ctx.enter_context(tc.tile_pool(name="outs", bufs=2))
        ps_1 = ctx.enter_context(tc.tile_pool(name="ps1", bufs=2, space="PSUM"))
        ps_2 = ctx.enter_context(tc.tile_pool(name="ps2", bufs=2, space="PSUM"))

        # ---- loads (spread across engine DMA queues so the first QKV
        # chunk's inputs land fast instead of serializing on one queue) ----
        wq_sb = []
        for k in range(KC):
            w = p_wq.tile([PT, 3 * NC_], bf16, tag="wq")
            nc.scalar.dma_start(w[:], wqkv_d[k * PT:(k + 1) * PT, :])
            wq_sb.append(w)

        bq_sb = []
        for m in range(MQKV):
            b = p_small.tile([PT, 1], f32, tag="bq", bufs=MQKV)
            nc.scalar.dma_start(b[:], bqkv_d[m * PT:(m + 1) * PT, :])
            bq_sb.append(b)

        # x^T column-chunked loads so qkv matmuls can start early
        xt_q = [nc.sync, nc.scalar, nc.gpsimd]
        xt_sb = [p_xt.tile([PT, T], bf16, tag="xt", name=f"xt{k}") for k in range(KC)]
        for cq in range(NQC):
            for k in range(KC):
                xt_q[(cq * KC + k) % 3].dma_start(
                    xt_sb[k][:, cq * QC:(cq + 1) * QC],
                    xT_d[k * PT:(k + 1) * PT, cq * QC:(cq + 1) * QC],
                )

        trimask = p_small.tile([PT, PT], bf16, tag="trimask")
        nc.scalar.dma_start(trimask[:], trimask_d[:])
        ident = p_small.tile([PT, PT], bf16, tag="ident")
        nc.scalar.dma_start(ident[:], ident_d[:])
        wo_sb = []
        for k in range(2):
            w = p_wo.tile([PT, C], bf16, tag="wo")
            nc.scalar.dma_start(w[:], wout_d[k * PT:(k + 1) * PT, :])
            wo_sb.append(w)
        bo_sb = []
        for e in range(C // PT):
            b = p_small.tile([PT, 1], f32, tag="bo", bufs=C // PT)
            nc.scalar.dma_start(b[:], bout_d[e * PT:(e + 1) * PT, :])
            bo_sb.append(b)

        qk_sb = [p_qk.tile([PT, T], bf16, tag="qk", name=f"qk{j}") for j in range(4)]
        va_sb = [p_va.tile([PT, HPC * VAP], bf16, tag="va", name=f"va{t}") for t in range(TT)]
        un_sb = [p_un.tile([PT, T], bf16, tag="un", name=f"un{j}") for j in range(2)]

        # ones columns for the softmax row-sum, filled once up front
        # (transposed V only ever writes the D v-columns per head)
        for t in range(TT):
            nc.gpsimd.dma_start(
                va_sb[t].rearrange("p (h v) -> p h v", v=VAP)[:, :, D:D + 1],
                ones4_d[:],
            )

        def qkv_chunk(cq):
            cs = slice(cq * QC, (cq + 1) * QC)
            for m in range(MQKV):
                ps = ps_1.tile([PT, QC], f32, tag="sa")
                for k in range(KC):
                    nc.tensor.matmul(
                        ps[:],
                        wq_sb[k][:, m * PT:(m + 1) * PT],
                        xt_sb[k][:, cs],
                        start=(k == 0), stop=(k == KC - 1),
                    )
                if m < 4:
                    nc.vector.tensor_scalar_add(qk_sb[m][:, cs], ps[:], bq_sb[m])
                else:
                    h0, h1 = 2 * (m - 4), 2 * (m - 4) + 1
                    for q4 in range(4):
                        t = cq * 4 + q4
                        vtp = p_vt.tile([PT, PT], bf16, tag="vt")
                        nc.vector.tensor_scalar_add(
                            vtp[:], ps[:, q4 * PT:(q4 + 1) * PT], bq_sb[m])
                        # V^T -> V via DMA-transpose XBAR (keeps PE/DVE free)
                        nc.sync.dma_start(
                            va_sb[t][:, h0 * VAP:h0 * VAP + D], vtp[0:D, :],
                            transpose=True)
                        nc.sync.dma_start(
                            va_sb[t][:, h1 * VAP:h1 * VAP + D], vtp[D:PT, :],
                            transpose=True)

        def attn_chunk(cq):
            cs = slice(cq * QC, (cq + 1) * QC)
            nts = 4 * cq + 4
            for j in range(2):          # head pair (2j, 2j+1)
                h0, h1 = 2 * j, 2 * j + 1
                psu = ps_2.tile([PT, 2 * QC], f32, tag="acc")

                def s_mm(t):
                    p = t - 4 * cq
                    s = max(p, 0) * PT
                    psS = ps_1.tile([PT, 2 * QC], f32, tag="sa")
                    tsl = slice(t * PT, (t + 1) * PT)
                    qsl = slice(cq * QC + s, (cq + 1) * QC)
                    nc.tensor.matmul(
                        psS[:, s:QC],
                        qk_sb[2 + j][0:D, tsl], qk_sb[j][0:D, qsl],
                        start=True, stop=True, tile_position=(0, 0),
                    )
                    nc.tensor.matmul(
                        psS[:, QC + s:2 * QC],
                        qk_sb[2 + j][D:PT, tsl], qk_sb[j][D:PT, qsl],
                        start=True, stop=True, tile_position=(D, 0),
                    )
                    return psS

                psS = s_mm(0)
                for t in range(nts):
                    p = t - 4 * cq      # >= 0 on diagonal-crossing tiles
                    s = max(p, 0) * PT  # skip fully-masked leading columns
                    pt = p_pt.tile([PT, 2 * QC], bf16, tag="pt")
                    pt3 = pt.rearrange("p (h w) -> p h w", h=2)
                    psS3 = psS.rearrange("p (h w) -> p h w", h=2)
                    nc.scalar.activation(pt3[:, :, s:QC], psS3[:, :, s:QC],
                                         Act.Exp, scale=0.125)
                    # issue next tile's S matmuls so the PE isn't idle
                    # while ACT computes this tile's exp
                    if t + 1 < nts:
                        psS_next = s_mm(t + 1)
                    if p >= 0:
                        nc.vector.tensor_mul(
                            pt[:, s:s + PT], pt[:, s:s + PT], trimask[:])
                        nc.vector.tensor_mul(
                            pt[:, QC + s:QC + s + PT], pt[:, QC + s:QC + s + PT],
                            trimask[:])
                    nc.tensor.matmul(
                        psu[0:VA, s:QC],
                        va_sb[t][:, h0 * VAP:h0 * VAP + VA], pt[:, s:QC],
                        start=(t == 0), stop=(t == nts - 1),
                    )
                    nc.tensor.matmul(
                        psu[0:VA, QC + s:2 * QC],
                        va_sb[t][:, h1 * VAP:h1 * VAP + VA], pt[:, QC + s:2 * QC],
                        start=(t == 0), stop=(t == nts - 1),
                    )
                    if t + 1 < nts:
                        psS = psS_next
                # normalize: rows 0..63 = U^T, row 64 = l (both heads).
                # l: psum row 64 -> SBUF (gpsimd reads PSUM) -> partition 0
                # (dma shift; partition_broadcast ucode reads physical
                # partition 0) -> broadcast -> approx-reciprocal at offset 0
                # (reciprocal_approx_fast misbehaves at partition offset 64)
                rr = p_norm.tile([VA, 2 * QC], f32, tag="rr")
                nc.vector.tensor_copy(rr[D:VA, :], psu[D:VA, :])
                rl = p_norm.tile([D, 2 * QC], f32, tag="rl")
                nc.gpsimd.dma_start(rl[0:1, :], rr[D:VA, :])
                nc.gpsimd.partition_broadcast(rl[0:D, :], rl[0:1, :])
                rb = p_norm.tile([D, 2 * QC], f32, tag="rb")
                nc.vector.reciprocal_approx_fast(rb[:], rl[:])
                nc.vector.tensor_mul(un_sb[j][0:D, cs], psu[0:D, 0:QC],
                                     rb[:, 0:QC])
                ut = p_norm.tile([D, QC], bf16, tag="ut")
                nc.vector.tensor_mul(ut[:], psu[0:D, QC:2 * QC], rb[:, QC:2 * QC])
                nc.gpsimd.dma_start(un_sb[j][D:PT, cs], ut[:])

        def outproj_chunk(cq):
            cs = slice(cq * QC, (cq + 1) * QC)
            for ep in range(4):
                pp2 = ps_1.tile([PT, 2 * QC], f32, tag="sa")
                for half in range(2):
                    e = 2 * ep + half
                    for k in range(2):
                        nc.tensor.matmul(
                            pp2[:, half * QC:(half + 1) * QC],
                            wo_sb[k][:, e * PT:(e + 1) * PT],
                            un_sb[k][:, cs],
                            start=(k == 0), stop=(k == 1),
                        )
                ot = p_out.tile([PT, 2 * QC], bf16, tag="ot")
                for half in range(2):
                    e = 2 * ep + half
                    nc.vector.tensor_scalar_add(
                        ot[:, half * QC:(half + 1) * QC],
                        pp2[:, half * QC:(half + 1) * QC], bo_sb[e])
                    nc.sync.dma_start(
                        outT_d[e * PT:(e + 1) * PT, cs],
                        ot[:, half * QC:(half + 1) * QC])

        # fused pipeline: out-proj trails attention by one chunk so the
        # PE never stalls on the normalize chain
        for cq in range(NQC):
            qkv_chunk(cq)
            attn_chunk(cq)
            if cq > 0:
                outproj_chunk(cq - 1)
        outproj_chunk(NQC - 1)

    nc.compile()
    return nc


def _get_nc():
    if "nc" not in _CACHE:
        _CACHE["nc"] = _build_nc()
    return _CACHE["nc"]


def _make_in_maps(x, W_qkv, b_qkv, W_out, b_out):
    import ml_dtypes

    bf16 = ml_dtypes.bfloat16
    x = np.asarray(x, dtype=np.float32)
    W_qkv = np.asarray(W_qkv, dtype=np.float32)
    b_qkv = np.asarray(b_qkv, dtype=np.float32)
    W_out = np.asarray(W_out, dtype=np.float32)
    b_out = np.asarray(b_out, dtype=np.float32)

    i = np.arange(PT)[:, None]
    j = np.arange(PT)[None, :]
    trimask = (i <= j).astype(bf16)
    ones4 = np.ones((PT, HPC), dtype=bf16)
    ident = np.eye(PT, dtype=bf16)

    in_maps = []
    for c in range(N_CORES):
        b, g = divmod(c, 4)
        gs = slice(g * NC_, (g + 1) * NC_)
        wqkv_c = np.ascontiguousarray(np.concatenate(
            [W_qkv[:, gs], W_qkv[:, C:][:, gs], W_qkv[:, 2 * C:][:, gs]],
            axis=1).astype(bf16))
        bqkv_c = np.ascontiguousarray(np.concatenate(
            [b_qkv[gs], b_qkv[C:][gs], b_qkv[2 * C:][gs]])[:, None])
        bout_c = (b_out if g == 0 else np.zeros_like(b_out))[:, None]
        in_maps.append({
            "xT": np.ascontiguousarray(x[b].T.astype(bf16)),
            "wqkv": wqkv_c,
            "bqkv": bqkv_c,
            "wout": np.ascontiguousarray(W_out[gs, :].astype(bf16)),
            "bout": np.ascontiguousarray(bout_c),
            "trimask": trimask,
            "ones4": ones4,
            "ident": ident,
        })
    return in_maps


def _assemble(results):
    out = np.empty((B, T, C), dtype=np.float32)
    for b in range(B):
        acc = results[4 * b]["outT"].astype(np.float32)
        for g in range(1, 4):
            acc += results[4 * b + g]["outT"].astype(np.float32)
        out[b] = acc.T
    return out


def kernel(x, W_qkv, b_qkv, W_out, b_out):
    from concourse import bass_utils
    nc = _get_nc()
    in_maps = _make_in_maps(x, W_qkv, b_qkv, W_out, b_out)
    res = bass_utils.run_bass_kernel_spmd(nc, in_maps, core_ids=list(range(N_CORES)))
    return _assemble(res.results)
